# revision 39
# baseline (speedup 1.0000x reference)
"""Multi-head causal attention (GQA + QK-RMSNorm + RoPE) for Trainium2, 8 cores.

Sharding: 8 cores = 2 batches x 4 head-groups (tensor parallel over heads).
Each core handles one batch and 8 Q heads + 2 KV heads:
  - QKV projection for its head slice (fp16 matmuls, fp32 psum)
  - QK RMSNorm + RoPE (gamma folded into host-side cos/sin tables)
  - causal attention in k-major (transposed-scores) layout:
      sT[k, q] = kT.T @ qT ; p = exp(sT/8) ; ctx = pT.T @ [v | 1]
    (the ones column yields the softmax denominator for free)
  - output projection against its w_out column slice -> partial [S, D] fp16
Host sums the 4 head-group partials per batch (fp32 accumulate).

Layout/schedule notes:
  - all DRAM inputs host-packed so every DMA moves >=4KB per partition in
    128 contiguous descriptors (~1us issue cost, full DMA bus rate)
  - fp16 rope tables (keeps DVE 2x element rate on the rope muls)
  - QKV runs one 512-token block ahead of attention; out-projection of
    older blocks drains inside attention's exp-bound c-loops as PE filler,
    with a cadence matched to each block's exp deficit
  - scores/exp trimmed to the causal triangle at 128-col granularity
  - rstd = exp(-0.5*ln(ms+eps)) and is applied after RoPE (they commute),
    so the one preloaded ACT table (ln/exp/square) serves every
    activation with no reloads, and the stat chain overlaps the rotation
  - in the repeat-timing variant the loop is software-pipelined: input
    DMAs for iteration k+1 issue during iteration k's attention tail, and
    the last block's out-projection drains at the next iteration's head
  - GPSIMD/Pool cannot touch PSUM on real hardware; all psum evacuation
    sits on ACT (phase 1) and DVE (transposes, out-proj staging)
"""
import sys
import numpy as np
from contextlib import ExitStack

if '/opt/trn_rl_repo' not in sys.path:
    sys.path.insert(0, '/opt/trn_rl_repo')

import concourse.bacc as bacc
import concourse.tile as tile
import concourse.mybir as mybir
from concourse.bass_utils import run_bass_kernel_spmd

dt = mybir.dt
AF = mybir.ActivationFunctionType
AX = mybir.AxisListType
ALU = mybir.AluOpType

HEAD_DIM = 64
NUM_Q_HEADS = 32
NUM_KV_HEADS = 8
ROPE_FREQ = 10000.0
EPS = 1e-6

B, S, D = 2, 2048, 2048
QH = 8            # q heads per core
KVH = 2           # kv heads per core
N_CORES = 8
ST = S // 128      # 16 token tiles of 128
NJ = S // 512      # 4 big q blocks of 512
NI2 = S // 256     # 8 x-load chunks of 256 tokens

F16 = dt.float16
F32 = dt.float32


def _build(repeat=1):
    nc = bacc.Bacc("TRN2", target_bir_lowering=False, debug=False,
                   num_devices=N_CORES)

    # host-packed inputs: partition-major, big contiguous runs per partition
    xh = nc.dram_tensor("xh", [128, NI2, 16, 256], F16, kind="ExternalInput").ap()
    wh = nc.dram_tensor("wh", [128, 16, 768], F16, kind="ExternalInput").ap()
    woh = nc.dram_tensor("woh", [128, 4, D], F16, kind="ExternalInput").ap()
    tabh = nc.dram_tensor("tabh", [128, 4, ST, HEAD_DIM], F16,
                          kind="ExternalInput").ap()   # cq, sq, ck, sk
    mih = nc.dram_tensor("mih", [128, 256], F16, kind="ExternalInput").ap()
    out = nc.dram_tensor("out", [S, D], F16, kind="ExternalOutput").ap()

    with tile.TileContext(nc) as tc, ExitStack() as ctx:
        perm = ctx.enter_context(tc.tile_pool(name="perm", bufs=1))

        # ---- persistent tiles ----
        w_all = perm.tile([128, 16, 768], F16, tag="w")
        wo_all = perm.tile([128, 4, D], F16, tag="wo")
        tab_all = perm.tile([128, 4, ST, HEAD_DIM], F16, tag="tab")
        mi = perm.tile([128, 256], F16, tag="mi")   # [mask | ident]
        teps = perm.tile([128, 1], F32, tag="eps")
        nc.vector.memset(teps[:], EPS)

        qT = perm.tile([128, 4, S], F16, tag="qT")      # [2head x 64d, hp, s]
        kT = perm.tile([128, KVH, S], F16, tag="kT")    # dup'd 64d halves
        vext = perm.tile([128, KVH, ST, HEAD_DIM + 1], F16, tag="vx")
        nc.vector.memset(vext[:, :, :, HEAD_DIM:HEAD_DIM + 1], 1.0)
        ctxT = perm.tile([128, 4, S], F16, tag="ctxT")
        nc.vector.memset(ctxT[:, :, 512 * 3:512 * 4], 0.0)
        ost = [perm.tile([128, D], F16, tag=f"ost{k}", name=f"ost{k}")
               for k in range(2)]

        # ---- pools (shared across phases) ----
        psA = ctx.enter_context(tc.tile_pool(name="psA", bufs=2, space="PSUM"))
        psB = ctx.enter_context(tc.tile_pool(name="psB", bufs=2, space="PSUM"))
        psP = ctx.enter_context(tc.tile_pool(name="psP", bufs=2, space="PSUM"))
        xpool = ctx.enter_context(tc.tile_pool(name="xpool", bufs=4))
        p1sb = ctx.enter_context(tc.tile_pool(name="p1sb", bufs=2))
        p1st = ctx.enter_context(tc.tile_pool(name="p1st", bufs=2))
        ptpool = ctx.enter_context(tc.tile_pool(name="ptpool", bufs=4))
        cnpool = ctx.enter_context(tc.tile_pool(name="cnpool", bufs=2))
        rcpool = ctx.enter_context(tc.tile_pool(name="rcpool", bufs=8))

        tmask = mi[:, 0:128]
        tident = mi[:, 128:256]

        # Preload the one ACT table that serves every activation we use
        # (exp, ln, square all live in natural_log_exp_and_others), so the
        # compile pass never needs to insert per-phase table reloads.
        from concourse.hw_specs import get_activation_tables
        _tables = list(get_activation_tables(nc.m.arch).items())
        _need = {AF.Exp, AF.Ln, AF.Square}
        _tid = next(i for i, (_, funcs) in enumerate(_tables)
                    if _need.issubset(funcs))
        nc.scalar.add_instruction(mybir.InstLoadActFuncSet(
            name=nc.get_next_instruction_name(),
            act_func_set_id=_tid, ins=[], outs=[]))

        # ---------------- DMA loads (per iteration) ----------------
        def load_xt(i2):
            t = xpool.tile([128, 16, 256], F16, tag="xt", name=f"xt{i2}")
            nc.sync.dma_start(t[:], xh[:, i2])
            return t

        def load_w(wc):
            nc.sync.dma_start(w_all[:, 4 * wc:4 * (wc + 1), :],
                              wh[:, 4 * wc:4 * (wc + 1), :])

        def load_misc():
            nc.sync.dma_start(tab_all[:], tabh[:])
            nc.sync.dma_start(mi[:], mih[:])
            nc.sync.dma_start(wo_all[:], woh[:])

        # prologue loads run once, before the repeat loop; the loop body
        # re-issues them at its tail for the next iteration (software
        # pipelining), so iteration starts never wait on input DMAs.
        xt_tiles = {}
        xt_tiles[0] = load_xt(0)
        load_w(0)
        load_w(1)
        load_w(2)
        xt_tiles[1] = load_xt(1)
        load_w(3)
        load_misc()

        rep_ctx = tc.For_i(0, repeat, 1) if repeat > 1 else None
        if rep_ctx is not None:
            ctx.enter_context(rep_ctx)
        wrap = repeat > 1

        # ---------------- out-projection (PE filler queue) ----------------
        po_queue = []

        def emit_po(i, qch):
            po = psP.tile([128, 512], F32, tag="po", name="po")
            for p in range(4):
                nc.tensor.matmul(
                    po[:], ctxT[:, p, 128 * i:128 * (i + 1)],
                    wo_all[:, p, 512 * qch:512 * (qch + 1)],
                    start=(p == 0), stop=(p == 3))
            nc.vector.tensor_copy(
                ost[i % 2][:, 512 * qch:512 * (qch + 1)], po[:])
            if qch == 3:
                nc.sync.dma_start(out[128 * i:128 * (i + 1), :], ost[i % 2][:])

        def pop_po(n=1):
            for _ in range(n):
                if po_queue:
                    emit_po(*po_queue.pop(0))

        # ---------------- phase 1: QKV + norm + rope ----------------
        def qkv_tile(i, xt):
            ii = i % 2
            pq = psA.tile([128, 512], F32, tag="pq",
                          padded_shape=[128, 1024], name="pq")
            pkv = psB.tile([128, 256], F32, tag="pkv",
                           padded_shape=[128, 260], name="pkv")
            for dc in range(16):
                xs = xt[:, dc, 128 * ii:128 * (ii + 1)]
                nc.tensor.matmul(pq[:], xs, w_all[:, dc, 0:512],
                                 start=(dc == 0), stop=(dc == 15))
                nc.tensor.matmul(pkv[:], xs, w_all[:, dc, 512:768],
                                 start=(dc == 0), stop=(dc == 15))
            qsb = p1sb.tile([128, 640], F16, tag="qsb")
            nc.scalar.copy(qsb[:, 0:512], pq[:])
            nc.scalar.copy(qsb[:, 512:640], pkv[:, 0:128])
            nc.vector.tensor_copy(vext[:, 0, i, 0:HEAD_DIM], pkv[:, 128:192])
            nc.vector.tensor_copy(vext[:, 1, i, 0:HEAD_DIM], pkv[:, 192:256])

            # RMSNorm stat chain (ACT square -> DVE reduce -> ACT ln/exp)
            # runs concurrently with RoPE on the raw projections; the
            # per-(token,head) rstd commutes with the rotation, so it is
            # applied once at the end.
            sqt = p1sb.tile([128, 640], F16, tag="sqt")
            nc.scalar.square(sqt[:], qsb[:])
            ssum = p1st.tile([128, 10], F32, tag="ssum")
            nc.vector.tensor_reduce(
                ssum[:], sqt[:].rearrange("p (h d) -> p h d", h=10),
                axis=AX.X, op=ALU.add)
            # rstd = exp(-0.5 * ln(ms + eps)); ln/exp/square share one ACT
            # table (natural_log_exp_and_others), so no table reloads vs Exp
            lnv = p1st.tile([128, 10], F32, tag="lnv")
            nc.scalar.activation(lnv[:], ssum[:], AF.Ln,
                                 bias=teps[:], scale=1.0 / HEAD_DIM)
            rstd = p1st.tile([128, 10], F16, tag="rstd")
            nc.scalar.activation(rstd[:], lnv[:], AF.Exp, scale=-0.5)
            qr = p1sb.tile([128, 640], F16, tag="qr")
            t2 = p1sb.tile([128, 640], F16, tag="t2")
            for part, nh, ti_c, ti_s in (("q", QH, 0, 1), ("k", KVH, 2, 3)):
                off = 0 if part == "q" else 512
                qs3 = qsb[:, off:off + 64 * nh].rearrange(
                    "p (h d) -> p h d", h=nh)
                qs4 = qsb[:, off:off + 64 * nh].rearrange(
                    "p (h two x) -> p h two x", h=nh, two=2)
                cosb = tab_all[:, ti_c, i, :].unsqueeze(1) \
                    .broadcast_to([128, nh, HEAD_DIM])
                sin4 = tab_all[:, ti_s, i, :].unsqueeze(1) \
                    .broadcast_to([128, nh, HEAD_DIM]) \
                    .rearrange("p h (two x) -> p h two x", two=2)
                t2v = t2[:, off:off + 64 * nh].rearrange(
                    "p (h two x) -> p h two x", h=nh, two=2)
                nc.vector.tensor_mul(t2v[:, :, 0, :], qs4[:, :, 1, :],
                                     sin4[:, :, 0, :])
                nc.vector.tensor_mul(t2v[:, :, 1, :], qs4[:, :, 0, :],
                                     sin4[:, :, 1, :])
                qr3 = qr[:, off:off + 64 * nh].rearrange(
                    "p (h d) -> p h d", h=nh)
                nc.vector.tensor_mul(qr3, qs3, cosb)
            nc.vector.tensor_add(qr[:], qr[:], t2[:])
            qn = p1sb.tile([128, 640], F16, tag="qn")
            nc.vector.tensor_mul(
                qn[:].rearrange("p (h d) -> p h d", h=10),
                qr[:].rearrange("p (h d) -> p h d", h=10),
                rstd[:].unsqueeze(2).broadcast_to([128, 10, HEAD_DIM]))
            return qn

        def p1_transposes(i, qr):
            tpq = psA.tile([128, 512], F16, tag="pq",
                           padded_shape=[128, 2048], name="tpq")
            for p in range(4):
                nc.tensor.transpose(tpq[:, 128 * p:128 * (p + 1)],
                                    qr[:, 128 * p:128 * (p + 1)], tident)
            nc.vector.tensor_copy(
                qT[:, :, 128 * i:128 * (i + 1)],
                tpq[:].rearrange("p (h x) -> p h x", h=4))
            tpk = psB.tile([128, 256], F16, tag="pkv",
                           padded_shape=[128, 520], name="tpk")
            for g in range(KVH):
                src = qr[:, 512 + 64 * g:512 + 64 * (g + 1)]
                nc.tensor.transpose(tpk[0:64, 128 * g:128 * (g + 1)],
                                    src, tident)
                nc.tensor.transpose(tpk[64:128, 128 * g:128 * (g + 1)],
                                    src, tident)
            nc.vector.tensor_copy(
                kT[:, :, 128 * i:128 * (i + 1)],
                tpk[:].rearrange("p (h x) -> p h x", h=KVH))

        # ---------------- attention block ----------------
        def attn_J(J, prelude=None):
            ctxn = [cnpool.tile([128, 512], F16, tag=f"cn{jj}", name=f"cn{jj}")
                    for jj in range(4)]
            pop_po(4)   # cover the last p1 chain's latency with PE work
            for hp in range(4):          # heads (2hp, 2hp+1), both use kv g
                if prelude is not None:
                    prelude(hp)          # qkv tile of block J+1 as PE filler
                g = hp // 2
                cbank = [psB.tile([128, 260], F32, tag="pkv", name=f"cb{w}")
                         for w in (0, 1)]

                def epi(jj):
                    w, loc = jj // 2, jj % 2
                    cb2 = cbank[w][:, 130 * loc:130 * (loc + 1)].rearrange(
                        "p (h e) -> p h e", h=2)
                    rc = rcpool.tile([128, 2], F32, tag="rc", name="rc")
                    nc.vector.reciprocal(rc[:], cb2[:, :, 64:65].squeeze(2))
                    nc.vector.tensor_mul(
                        ctxn[jj][:, 128 * hp:128 * (hp + 1)].rearrange(
                            "p (h d) -> p h d", h=2),
                        cb2[:, :, 0:64],
                        rc[:].unsqueeze(2).broadcast_to([128, 2, HEAD_DIM]))

                def ctx_mms(pt, jj0, c):
                    for jj in range(jj0, 4):
                        w, loc = jj // 2, jj % 2
                        for hh in (0, 1):
                            o = 130 * loc + 65 * hh
                            nc.tensor.matmul(
                                cbank[w][:, o:o + 65],
                                pt[:, 512 * hh + 128 * jj:512 * hh + 128 * (jj + 1)],
                                vext[:, g, c, :],
                                start=(c == 0 and loc == 0 and hh == 0),
                                stop=(c == 4 * J + jj and jj % 2 == 1 and hh == 1),
                                skip_group_check=True)

                pending = None
                for c in range(4 * J + 4):
                    jj0 = max(0, c - 4 * J)
                    sT2 = psA.tile([128, 1024], F32, tag="pq", name="sT2")
                    for hh in (0, 1):
                        nc.tensor.matmul(
                            sT2[:, 512 * hh + 128 * jj0:512 * (hh + 1)],
                            kT[64 * hh:64 * hh + 64, g, 128 * c:128 * (c + 1)],
                            qT[64 * hh:64 * hh + 64, hp,
                               512 * J + 128 * jj0:512 * (J + 1)],
                            start=True, stop=True)
                    if pending is not None:
                        ctx_mms(*pending)
                        pending = None
                        if c == 4 * J + 2:   # bank 0 (jj 0,1) is complete
                            epi(0)
                            epi(1)
                    # PE filler cadence matched to each block's exp deficit
                    if J >= 2 or c % 2 == 1:
                        pop_po(1)
                    pt = ptpool.tile([128, 1024], F16, tag="pt", name="pt")
                    ptv = pt[:].rearrange("p (h x) -> p h x", h=2)[:, :, 128 * jj0:512]
                    sTv = sT2[:].rearrange("p (h x) -> p h x", h=2)[:, :, 128 * jj0:512]
                    nc.scalar.activation(ptv, sTv, AF.Exp, scale=0.125)
                    if c >= 4 * J:      # diagonal: triangular mask, both heads
                        dv = pt[:].rearrange("p (h x) -> p h x", h=2)[
                            :, :, 128 * jj0:128 * (jj0 + 1)]
                        nc.vector.tensor_mul(
                            dv, dv,
                            tmask.unsqueeze(1).broadcast_to([128, 2, 128]))
                    pending = (pt, jj0, c)
                ctx_mms(*pending)
                pending = None
                for jj in (2, 3):
                    epi(jj)
            for jj in range(4):
                tpc = psA.tile([128, 512], F16, tag="pq",
                               padded_shape=[128, 2048], name="tpc")
                for p in range(4):
                    nc.tensor.transpose(
                        tpc[:, 128 * p:128 * (p + 1)],
                        ctxn[jj][:, 128 * p:128 * (p + 1)], tident)
                nc.vector.tensor_copy(
                    ctxT[:, :, 512 * J + 128 * jj:512 * J + 128 * (jj + 1)],
                    tpc[:].rearrange("p (h x) -> p h x", h=4))
            # defer out-proj; drained inside the next J's c-loop
            for i in range(4 * J, 4 * J + 4):
                for qch in range(4):
                    po_queue.append((i, qch))

        # ---------------- main schedule ----------------
        # qkv runs one block ahead of attention; block J+1's four token
        # tiles are interleaved into attn(J)'s four per-head-pair loops so
        # the exp-bound c-loops always have PE work queued behind them.
        p1_pending = None

        def do_tile(i):
            nonlocal p1_pending
            t = i % 4
            b = i // 4
            if t == 0 and 2 * b + 2 < NI2:
                xt_tiles[2 * b + 2] = load_xt(2 * b + 2)
            if t == 1 and 2 * b + 3 < NI2:
                xt_tiles[2 * b + 3] = load_xt(2 * b + 3)
            qr = qkv_tile(i, xt_tiles[i // 2])
            if p1_pending is not None:
                p1_transposes(*p1_pending)
            p1_pending = (i, qr)

        def flush_p1():
            nonlocal p1_pending
            if p1_pending is not None:
                p1_transposes(*p1_pending)
                p1_pending = None

        for i in range(4):
            do_tile(i)
        flush_p1()
        if wrap:
            # software-pipelined wrap: the previous iteration's block-3
            # out-projection drains here (ctxT rows persist; every
            # iteration computes identical values)
            for i in range(12, 16):
                for qch in range(4):
                    po_queue.append((i, qch))
        for J in range(NJ):
            if J + 1 < NJ:
                for i in range(4 * (J + 1), 4 * (J + 1) + 4):
                    do_tile(i)
                flush_p1()
            attn_J(J)
            if wrap and J == 2:
                # reload inputs for the next iteration while attn(3) runs
                for wc in range(4):
                    load_w(wc)
                nc.sync.dma_start(tab_all[:], tabh[:])
                xt_tiles[0] = load_xt(0)
                xt_tiles[1] = load_xt(1)
        if wrap:
            nc.sync.dma_start(mi[:], mih[:])
            nc.sync.dma_start(wo_all[:], woh[:])
            po_queue.clear()   # block-3 drains at the next iteration's head
        else:
            while po_queue:
                emit_po(*po_queue.pop(0))

    nc.compile()
    return nc


_NC = {}


def _get_nc(repeat=1):
    if repeat not in _NC:
        _NC[repeat] = _build(repeat)
    return _NC[repeat]


_RUNNER = {}


def _get_runner(repeat=1):
    """Build (once) a jitted 8-core sharded callable around the bass program.

    Slim replica of bass2jax.run_bass_via_pjrt's multi-core path, kept
    reusable so repeated invocations skip retracing/recompilation.
    """
    if repeat in _RUNNER:
        return _RUNNER[repeat]
    import jax
    from jax.sharding import Mesh, PartitionSpec
    from jax.experimental.shard_map import shard_map
    from concourse import bass2jax
    from concourse import mybir as _mybir

    nc = _get_nc(repeat)
    bass2jax.install_neuronx_cc_hook()

    partition_name = nc.partition_id_tensor.name if nc.partition_id_tensor else None
    in_names, out_names, out_avals, zero_outs = [], [], [], []
    for alloc in nc.m.functions[0].allocations:
        if not isinstance(alloc, _mybir.MemoryLocationSet):
            continue
        name = alloc.memorylocations[0].name
        if alloc.kind == "ExternalInput":
            if name != partition_name:
                in_names.append(name)
        elif alloc.kind == "ExternalOutput":
            shape = tuple(alloc.tensor_shape)
            np_dt = _mybir.dt.np(alloc.dtype)
            out_names.append(name)
            out_avals.append(jax.core.ShapedArray(shape, np_dt))
            zero_outs.append(np.zeros(shape, np_dt))
    n_params = len(in_names)
    all_in_names = list(in_names) + list(out_names)
    if partition_name is not None:
        all_in_names.append(partition_name)

    def _body(*args):
        operands = list(args)
        if partition_name is not None:
            operands.append(bass2jax.partition_id_tensor())
        outs = bass2jax._bass_exec_p.bind(
            *operands,
            out_avals=tuple(out_avals),
            in_names=tuple(all_in_names),
            out_names=tuple(out_names),
            lowering_input_output_aliases=(),
            sim_require_finite=True,
            sim_require_nnan=True,
            nc=nc,
        )
        return tuple(outs)

    devices = jax.devices()[:N_CORES]
    mesh = Mesh(np.asarray(devices), ("core",))
    in_specs = (PartitionSpec("core"),) * (n_params + len(out_names))
    out_specs = (PartitionSpec("core"),) * len(out_names)
    sharded = jax.jit(shard_map(_body, mesh=mesh, in_specs=in_specs,
                                out_specs=out_specs, check_rep=False),
                      keep_unused=True)

    concat_zeros = [np.zeros((N_CORES * z.shape[0], *z.shape[1:]), z.dtype)
                    for z in zero_outs]

    _dev_cache = {}

    def run(in_maps, iters=1, time_list=None, fetch=True):
        import time as _time
        from jax.sharding import NamedSharding
        shard = NamedSharding(mesh, PartitionSpec("core"))
        key = id(in_maps)
        if key not in _dev_cache:
            per_core = [[np.asarray(m[nm]) for nm in in_names] for m in in_maps]
            concat_in = [np.concatenate([per_core[c][i] for c in range(N_CORES)],
                                        axis=0) for i in range(n_params)]
            dev_in = [jax.device_put(a, shard) for a in concat_in]
            dev_zero = [jax.device_put(z, shard) for z in concat_zeros]
            jax.block_until_ready(dev_in)
            _dev_cache.clear()
            _dev_cache[key] = (dev_in, dev_zero)
        dev_in, dev_zero = _dev_cache[key]
        out_arrs = None
        if iters <= 1:
            out_arrs = sharded(*dev_in, *dev_zero)
            jax.block_until_ready(out_arrs)
        else:
            # async batch: submit all, block once; caller computes slope
            sharded(*dev_in, *dev_zero)  # warm
            t0 = _time.perf_counter()
            for _ in range(iters):
                out_arrs = sharded(*dev_in, *dev_zero)
            jax.block_until_ready(out_arrs)
            if time_list is not None:
                time_list.append(_time.perf_counter() - t0)
        if not fetch:
            del out_arrs
            return None
        return [
            {nm: np.asarray(out_arrs[i]).reshape(N_CORES, *out_avals[i].shape)[c]
             for i, nm in enumerate(out_names)}
            for c in range(N_CORES)
        ]

    _RUNNER[repeat] = run
    return run


def _host_tables(q_gamma, k_gamma):
    pos = np.arange(S, dtype=np.float32)
    inv = 1.0 / (ROPE_FREQ ** (np.arange(0, HEAD_DIM, 2, dtype=np.float32)
                               / HEAD_DIM))
    fr = pos[:, None] * inv[None, :]
    emb = np.concatenate([fr, fr], axis=-1)
    cos = np.cos(emb).astype(np.float32)
    sin = np.sin(emb).astype(np.float32)
    outs = []
    for gamma in (q_gamma, k_gamma):
        g = gamma.astype(np.float32)
        cos_g = cos * g[None, :]
        sin_eff = np.concatenate([-sin[:, :32] * g[None, 32:],
                                  sin[:, 32:] * g[None, :32]], axis=-1)
        outs += [cos_g, sin_eff]
    return outs  # cos_q, sin_q, cos_k, sin_k


def _make_in_maps(x, w_qkv, w_out, q_gamma, k_gamma):
    cos_q, sin_q, cos_k, sin_k = _host_tables(q_gamma, k_gamma)
    # [4, S, 64] -> [128, 4, ST, 64]  (s = 128*i + p)
    tabs4 = np.stack([cos_q, sin_q, cos_k, sin_k])
    tabh = np.ascontiguousarray(
        tabs4.reshape(4, ST, 128, HEAD_DIM).transpose(2, 0, 1, 3)
    ).astype(np.float16)
    mask = (np.arange(128)[None, :] >= np.arange(128)[:, None]).astype(np.float16)
    ident = np.eye(128, dtype=np.float16)
    mih = np.ascontiguousarray(np.concatenate([mask, ident], axis=1))

    in_maps = []
    for core in range(N_CORES):
        b, g = core // 4, core % 4
        # x: [S, D] -> xT [D, S] -> [128, NI2, 16, 256]  (d = 128*dc + p)
        xT = x[b].T.astype(np.float16)
        xhc = np.ascontiguousarray(
            xT.reshape(16, 128, NI2, 256).transpose(1, 2, 0, 3))
        wq = w_qkv[512 * g:512 * (g + 1)]                      # 8 q heads
        wk = w_qkv[2048 + 128 * g:2048 + 128 * (g + 1)]        # 2 k heads
        wv = w_qkv[2560 + 128 * g:2560 + 128 * (g + 1)]        # 2 v heads
        wqkvT = np.concatenate([wq, wk, wv], axis=0).T.astype(np.float16)
        whc = np.ascontiguousarray(
            wqkvT.reshape(16, 128, 768).transpose(1, 0, 2))
        woutT = w_out[:, 512 * g:512 * (g + 1)].T.astype(np.float16)
        wohc = np.ascontiguousarray(
            woutT.reshape(4, 128, D).transpose(1, 0, 2))
        in_maps.append({
            "xh": xhc, "wh": whc, "woh": wohc, "tabh": tabh, "mih": mih,
        })
    return in_maps


def kernel(x, w_qkv, w_out, q_gamma, k_gamma):
    x = np.asarray(x)
    w_qkv = np.asarray(w_qkv)
    w_out = np.asarray(w_out)
    q_gamma = np.asarray(q_gamma)
    k_gamma = np.asarray(k_gamma)
    in_maps = _make_in_maps(x, w_qkv, w_out, q_gamma, k_gamma)
    results = _get_runner()(in_maps)
    out = np.empty((B, S, D), dtype=np.float32)
    for b in range(B):
        acc = results[4 * b]["out"].astype(np.float32)
        for g in range(1, 4):
            acc += results[4 * b + g]["out"].astype(np.float32)
        out[b] = acc
    return out
